# revision 2
# baseline (speedup 1.0000x reference)
"""Attention layer kernel for Trainium2, data-parallel over batch on 8 NeuronCores.

Reference computation (per full inputs):
    x          = input @ W.T                      [B, D]
    scores     = einsum('bsd,bd->bs', hids, x) - 100*(1-seg)
    attn       = softmax(scores, axis=1)          [B, S]
    ctx        = einsum('bs,bsd->bd', attn, hids) [B, D]
    returns (ctx, attn)

Sharding: batch B=32 split 4-per-core across 8 cores; W replicated
(pre-transposed on host so the contraction dim lands on SBUF partitions).

Per-core algorithm (flash-decoding style, reads source_hids exactly once):
  - x = input @ W^T on TensorE, broadcast to 128 partitions via a DRAM
    round-trip with a stride-0 partition AP.
  - scores per 128-row chunk via one fused DVE tensor_tensor_reduce
    (multiply by broadcast x, reduce over d, init with the -100*(1-seg) mask).
  - online softmax over blocks of chunks: running max M / sum Z, context
    accumulator C rescaled by alpha = exp(M_old - M_new) using a K=1 matmul
    into the same PSUM accumulation group as the per-chunk attn@hids matmuls.
  - s-index mapping s = p*NCH + n keeps every DMA contiguous (no transposes).
"""

import numpy as np

B, S, DIN, DOUT = 32, 4096, 1024, 1024
D = DIN
NCORES = 8
BLOC = B // NCORES  # batches per core
P = 128

_NC_CACHE = {}
LAST_RESULT = None  # BassKernelResults of the most recent run (for profiling)


def _build(s_total=S, blkc=8):
    """Build + compile the per-core Bass module. Same program on all cores."""
    import concourse.bacc as bacc
    import concourse.bass as bass
    import concourse.mybir as mybir
    import concourse.tile as tile
    from concourse.bass_isa import ReduceOp

    f32 = mybir.dt.float32
    Alu = mybir.AluOpType
    Act = mybir.ActivationFunctionType
    AxisX = mybir.AxisListType.X

    NCH = s_total // P          # score chunks per batch (columns of score tile)
    NBLK = NCH // blkc          # online-softmax blocks per batch
    KCH = D // P                # contraction chunks for the x projection
    NHALF = D // 512            # 512-wide matmul column tiles

    nc = bacc.Bacc("TRN2", target_bir_lowering=False, debug=False)

    inputT = nc.dram_tensor("inputT", [D, BLOC], f32, kind="ExternalInput").ap()
    hids = nc.dram_tensor("hids", [BLOC, s_total, D], f32, kind="ExternalInput").ap()
    seg = nc.dram_tensor("seg", [BLOC, s_total], f32, kind="ExternalInput").ap()
    WT = nc.dram_tensor("WT", [D, D], f32, kind="ExternalInput").ap()
    ctx_o = nc.dram_tensor("ctx", [BLOC, D], f32, kind="ExternalOutput").ap()
    attn_o = nc.dram_tensor("attn", [BLOC, s_total], f32, kind="ExternalOutput").ap()
    x_dram = nc.dram_tensor("x_scratch", [BLOC, D], f32, kind="Internal").ap()

    with tile.TileContext(nc) as tc:
        with (
            tc.tile_pool(name="big", bufs=3) as bigpool,
            tc.tile_pool(name="persist", bufs=1) as persist,
            tc.tile_pool(name="scores", bufs=BLOC) as scorepool,
            tc.tile_pool(name="small", bufs=24) as small,
            tc.tile_pool(name="pblk", bufs=4) as pblkpool,
            tc.tile_pool(name="cpool", bufs=4) as cpool,
            tc.tile_pool(name="outp", bufs=2) as outpool,
            tc.tile_pool(name="psA", bufs=1, space="PSUM") as psA,
            tc.tile_pool(name="psC", bufs=2, space="PSUM") as psC,
        ):
            # ---------------- prologue: x = input @ W^T, then broadcast ------
            WT_sb = persist.tile([P, KCH, D], f32, tag="WT")
            nc.sync.dma_start(out=WT_sb, in_=WT.rearrange("(k p) o -> p k o", p=P))
            inT_sb = persist.tile([P, KCH, BLOC], f32, tag="inT")
            nc.sync.dma_start(out=inT_sb, in_=inputT.rearrange("(k p) b -> p k b", p=P))

            psum_x = psA.tile([BLOC, D], f32, tag="psx")
            for n in range(NHALF):
                nsl = slice(n * 512, (n + 1) * 512)
                for k in range(KCH):
                    nc.tensor.matmul(
                        psum_x[:, nsl],
                        lhsT=inT_sb[:, k, :],
                        rhs=WT_sb[:, k, nsl],
                        start=(k == 0),
                        stop=(k == KCH - 1),
                    )
            x_sb = outpool.tile([BLOC, D], f32, tag="x_sb")
            nc.scalar.copy(x_sb, psum_x)
            nc.sync.dma_start(out=x_dram, in_=x_sb)

            # broadcast each batch's x row across all 128 partitions
            xb_sb = persist.tile([P, BLOC, D], f32, tag="xb")
            for b in range(BLOC):
                row = x_dram[b]
                src = bass.AP(
                    tensor=row.tensor,
                    offset=row.offset,
                    ap=[[0, P]] + [list(e) for e in row.ap],
                )
                nc.gpsimd.dma_start(out=xb_sb[:, b, :], in_=src)

            # additive mask term 100*seg - 100, laid out like the score tiles
            segadj = persist.tile([P, BLOC, NCH], f32, tag="segadj")
            for b in range(BLOC):
                seg_raw = outpool.tile([P, NCH], f32, tag="segraw")
                nc.sync.dma_start(
                    out=seg_raw, in_=seg[b].rearrange("(p n) -> p n", n=NCH)
                )
                nc.scalar.activation(
                    segadj[:, b, :], seg_raw, Act.Copy, bias=-100.0, scale=100.0
                )

            # ---------------- main loop: online softmax + context ------------
            for b in range(BLOC):
                scores = scorepool.tile([P, NCH], f32, tag="scores")
                m_cur = None
                z_cur = None
                c_cur = None
                for blk in range(NBLK):
                    hb = bigpool.tile([P, blkc, D], f32, tag="hids")
                    nc.sync.dma_start(
                        out=hb,
                        in_=hids[b].rearrange("(p n) d -> p n d", n=NCH)[
                            :, blk * blkc : (blk + 1) * blkc, :
                        ],
                    )
                    dummy = small.tile([P, 1], f32, tag="dummy")
                    rawb = pblkpool.tile([P, blkc], f32, tag="rawb")
                    for c in range(blkc):
                        nc.vector.affine_mul_reduce(
                            out=dummy.broadcast_to((P, D)),
                            accum_out=rawb[:, c : c + 1],
                            in0=hb[:, c, :],
                            in1=xb_sb[:, b, :],
                            scale=1.0,
                            bias=0.0,
                        )
                    blk_sl = scores[:, blk * blkc : (blk + 1) * blkc]
                    nc.vector.tensor_add(
                        blk_sl, rawb, segadj[:, b, blk * blkc : (blk + 1) * blkc]
                    )
                    mb = small.tile([P, 1], f32, tag="mb")
                    nc.vector.reduce_max(mb, blk_sl, axis=AxisX)
                    mb_all = small.tile([P, 1], f32, tag="mba")
                    nc.gpsimd.partition_all_reduce(mb_all, mb, P, ReduceOp.max)
                    if blk == 0:
                        m_new = mb_all
                        alpha = None
                    else:
                        m_new = small.tile([P, 1], f32, tag="mnew")
                        nc.vector.tensor_max(m_new, m_cur, mb_all)
                        diff = small.tile([P, 1], f32, tag="diff")
                        nc.vector.tensor_sub(diff, m_cur, m_new)
                        alpha = small.tile([P, 1], f32, tag="alpha")
                        nc.scalar.activation(alpha, diff, Act.Exp)
                    negm = small.tile([P, 1], f32, tag="negm")
                    nc.vector.tensor_scalar_mul(negm, m_new, -1.0)

                    pb = pblkpool.tile([P, blkc], f32, tag="pb")
                    zb = small.tile([P, 1], f32, tag="zb")
                    nc.scalar.activation(
                        pb, blk_sl, Act.Exp, bias=negm, accum_out=zb
                    )
                    zb_all = small.tile([P, 1], f32, tag="zba")
                    nc.gpsimd.partition_all_reduce(zb_all, zb, P, ReduceOp.add)
                    if blk == 0:
                        z_new = zb_all
                    else:
                        z_new = small.tile([P, 1], f32, tag="znew")
                        nc.vector.scalar_tensor_tensor(
                            out=z_new,
                            in0=z_cur,
                            scalar=alpha,
                            in1=zb_all,
                            op0=Alu.mult,
                            op1=Alu.add,
                        )

                    psum_c = psC.tile([1, D], f32, tag="psC")
                    for n in range(NHALF):
                        nsl = slice(n * 512, (n + 1) * 512)
                        if blk > 0:
                            nc.tensor.matmul(
                                psum_c[:, nsl],
                                lhsT=alpha[0:1, 0:1],
                                rhs=c_cur[:, nsl],
                                start=True,
                                stop=False,
                            )
                        for c in range(blkc):
                            nc.tensor.matmul(
                                psum_c[:, nsl],
                                lhsT=pb[:, c : c + 1],
                                rhs=hb[:, c, nsl],
                                start=(blk == 0 and c == 0),
                                stop=(c == blkc - 1),
                            )

                    if blk < NBLK - 1:
                        c_new = cpool.tile([1, D], f32, tag="C")
                        nc.scalar.copy(c_new, psum_c)
                        c_cur = c_new
                    else:
                        # finalize: ctx = C / Z, attn = exp(scores - M - ln Z)
                        rz = small.tile([P, 1], f32, tag="rz")
                        nc.vector.reciprocal(rz, z_new)
                        ctx_sb = outpool.tile([1, D], f32, tag="ctxsb")
                        nc.scalar.activation(
                            ctx_sb, psum_c, Act.Copy, scale=rz[0:1, 0:1]
                        )
                        nc.sync.dma_start(out=ctx_o[b : b + 1, :], in_=ctx_sb)

                        lnz = small.tile([P, 1], f32, tag="lnz")
                        nc.scalar.activation(lnz, z_new, Act.Ln)
                        nbias = small.tile([P, 1], f32, tag="nbias")
                        nc.vector.scalar_tensor_tensor(
                            out=nbias,
                            in0=m_new,
                            scalar=-1.0,
                            in1=lnz,
                            op0=Alu.mult,
                            op1=Alu.subtract,
                        )
                        attn_sb = outpool.tile([P, NCH], f32, tag="attnsb")
                        nc.scalar.activation(attn_sb, scores, Act.Exp, bias=nbias)
                        nc.sync.dma_start(
                            out=attn_o[b].rearrange("(p n) -> p n", n=NCH),
                            in_=attn_sb,
                        )
                    m_cur = m_new
                    z_cur = z_new

    nc.compile()
    return nc


def _get_nc(s_total=S, blkc=8):
    key = (s_total, blkc)
    if key not in _NC_CACHE:
        _NC_CACHE[key] = _build(s_total, blkc)
    return _NC_CACHE[key]


def make_in_maps(input, source_hids, seg_scores, W, s_total=S):
    input = np.ascontiguousarray(np.asarray(input, dtype=np.float32))
    source_hids = np.asarray(source_hids, dtype=np.float32)
    seg_scores = np.asarray(seg_scores, dtype=np.float32)
    W = np.asarray(W, dtype=np.float32)
    wt = np.ascontiguousarray(W.T)
    inT = np.ascontiguousarray(input.T)  # [D, B]
    in_maps = []
    for core in range(NCORES):
        sl = slice(core * BLOC, (core + 1) * BLOC)
        in_maps.append(
            {
                "inputT": np.ascontiguousarray(inT[:, sl]),
                "hids": np.ascontiguousarray(source_hids[sl]),
                "seg": np.ascontiguousarray(seg_scores[sl]),
                "WT": wt,
            }
        )
    return in_maps


def kernel(input, source_hids, seg_scores, W):
    global LAST_RESULT
    import os

    from concourse.bass_utils import run_bass_kernel_spmd

    s_total = np.asarray(source_hids).shape[1]
    nc = _get_nc(s_total=s_total)
    in_maps = make_in_maps(input, source_hids, seg_scores, W, s_total=s_total)
    trace = bool(int(os.environ.get("KERNEL_TRACE", "0")))
    res = run_bass_kernel_spmd(
        nc, in_maps, core_ids=list(range(NCORES)), trace=trace
    )
    LAST_RESULT = res
    ctx = np.concatenate([res.results[c]["ctx"] for c in range(NCORES)], axis=0)
    attn = np.concatenate([res.results[c]["attn"] for c in range(NCORES)], axis=0)
    return ctx, attn


# revision 5
# speedup vs baseline: 80.2988x; 80.2988x over previous
"""Attention layer kernel for Trainium2, data-parallel over batch on 8 NeuronCores.

Reference computation (per full inputs):
    x          = input @ W.T                      [B, D]
    scores     = einsum('bsd,bd->bs', hids, x) - 100*(1-seg)
    attn       = softmax(scores, axis=1)          [B, S]
    ctx        = einsum('bs,bsd->bd', attn, hids) [B, D]
    returns (ctx, attn)

Sharding: batch B=32 split 4-per-core across 8 cores; W replicated
(pre-transposed on host so the contraction dim lands on SBUF partitions).

Per-core algorithm (flash-decoding style, reads source_hids exactly once):
  - x = input @ W^T on TensorE, broadcast to 128 partitions via a DRAM
    round-trip with a stride-0 partition AP.
  - scores per 128-row chunk via one fused DVE tensor_tensor_reduce
    (multiply by broadcast x, reduce over d, init with the -100*(1-seg) mask).
  - online softmax over blocks of chunks: running max M / sum Z, context
    accumulator C rescaled by alpha = exp(M_old - M_new) using a K=1 matmul
    into the same PSUM accumulation group as the per-chunk attn@hids matmuls.
  - s-index mapping s = p*NCH + n keeps every DMA contiguous (no transposes).
"""

import numpy as np

B, S, DIN, DOUT = 32, 4096, 1024, 1024
D = DIN
NCORES = 8
BLOC = B // NCORES  # batches per core
P = 128

_NC_CACHE = {}
LAST_RESULT = None  # BassKernelResults of the most recent run (for profiling)


def _build(s_total=S, blkc=8, reps=1):
    """Build + compile the per-core Bass module. Same program on all cores.

    reps > 1 wraps the main computation in an on-device For_i loop — used
    only for timing (slope between reps removes host/RPC overhead).
    """
    import contextlib

    import concourse.bacc as bacc
    import concourse.bass as bass
    import concourse.mybir as mybir
    import concourse.tile as tile
    from concourse.bass_isa import ReduceOp

    f32 = mybir.dt.float32
    Alu = mybir.AluOpType
    Act = mybir.ActivationFunctionType
    AxisX = mybir.AxisListType.X

    NCH = s_total // P          # score chunks per batch (columns of score tile)
    NBLK = NCH // blkc          # online-softmax blocks per batch
    KCH = D // P                # contraction chunks for the x projection
    NHALF = D // 512            # 512-wide matmul column tiles

    nc = bacc.Bacc("TRN2", target_bir_lowering=False, debug=False)

    inputT = nc.dram_tensor("inputT", [D, BLOC], f32, kind="ExternalInput").ap()
    hids = nc.dram_tensor("hids", [BLOC, s_total, D], f32, kind="ExternalInput").ap()
    seg = nc.dram_tensor("seg", [BLOC, s_total], f32, kind="ExternalInput").ap()
    WT = nc.dram_tensor("WT", [D, D], f32, kind="ExternalInput").ap()
    ctx_o = nc.dram_tensor("ctx", [BLOC, D], f32, kind="ExternalOutput").ap()
    attn_o = nc.dram_tensor("attn", [BLOC, s_total], f32, kind="ExternalOutput").ap()
    x_dram = nc.dram_tensor("x_scratch", [BLOC, D], f32, kind="Internal").ap()

    with tile.TileContext(nc) as tc:
        with (
            tc.tile_pool(name="big", bufs=3) as bigpool,
            tc.tile_pool(name="persist", bufs=1) as persist,
            tc.tile_pool(name="scores", bufs=BLOC) as scorepool,
            tc.tile_pool(name="small", bufs=24) as small,
            tc.tile_pool(name="pblk", bufs=4) as pblkpool,
            tc.tile_pool(name="cpool", bufs=4) as cpool,
            tc.tile_pool(name="outp", bufs=2) as outpool,
            tc.tile_pool(name="psA", bufs=1, space="PSUM") as psA,
            tc.tile_pool(name="psC", bufs=2, space="PSUM") as psC,
        ):
            # ---------------- prologue: x = input @ W^T, then broadcast ------
            WT_sb = persist.tile([P, KCH, D], f32, tag="WT")
            nc.sync.dma_start(out=WT_sb, in_=WT.rearrange("(k p) o -> p k o", p=P))
            inT_sb = persist.tile([P, KCH, BLOC], f32, tag="inT")
            nc.sync.dma_start(out=inT_sb, in_=inputT.rearrange("(k p) b -> p k b", p=P))

            psum_x = psA.tile([BLOC, D], f32, tag="psx")
            for n in range(NHALF):
                nsl = slice(n * 512, (n + 1) * 512)
                for k in range(KCH):
                    nc.tensor.matmul(
                        psum_x[:, nsl],
                        lhsT=inT_sb[:, k, :],
                        rhs=WT_sb[:, k, nsl],
                        start=(k == 0),
                        stop=(k == KCH - 1),
                    )
            x_sb = outpool.tile([BLOC, D], f32, tag="x_sb")
            nc.scalar.copy(x_sb, psum_x)
            nc.sync.dma_start(out=x_dram, in_=x_sb)

            # broadcast each batch's x row across all 128 partitions
            xb_sb = persist.tile([P, BLOC, D], f32, tag="xb")
            for b in range(BLOC):
                row = x_dram[b]
                src = bass.AP(
                    tensor=row.tensor,
                    offset=row.offset,
                    ap=[[0, P]] + [list(e) for e in row.ap],
                )
                nc.gpsimd.dma_start(out=xb_sb[:, b, :], in_=src)

            # additive mask term 100*seg - 100, laid out like the score tiles
            segadj = persist.tile([P, BLOC, NCH], f32, tag="segadj")
            for b in range(BLOC):
                seg_raw = outpool.tile([P, NCH], f32, tag="segraw")
                nc.sync.dma_start(
                    out=seg_raw, in_=seg[b].rearrange("(p n) -> p n", n=NCH)
                )
                nc.scalar.activation(
                    segadj[:, b, :], seg_raw, Act.Copy, bias=-100.0, scale=100.0
                )

            # ---------------- main loop: online softmax + context ------------
            rep_cm = tc.For_i(0, reps, 1) if reps > 1 else contextlib.nullcontext()
            with rep_cm:
                _main_body(
                    nc, tc, bass, mybir, ReduceOp,
                    hids, seg, ctx_o, attn_o,
                    xb_sb, segadj,
                    bigpool, scorepool, small, pblkpool, cpool, outpool, psC,
                    s_total, blkc,
                )

    nc.compile()
    return nc


def _main_body(
    nc, tc, bass, mybir, ReduceOp,
    hids, seg, ctx_o, attn_o,
    xb_sb, segadj,
    bigpool, scorepool, small, pblkpool, cpool, outpool, psC,
    s_total, blkc,
):
    f32 = mybir.dt.float32
    Alu = mybir.AluOpType
    Act = mybir.ActivationFunctionType
    AxisX = mybir.AxisListType.X
    D = DIN
    NCH = s_total // P
    NBLK = NCH // blkc
    NHALF = D // 512
    if True:
            for b in range(BLOC):
                scores = scorepool.tile([P, NCH], f32, tag="scores")
                m_cur = None
                z_cur = None
                c_cur = None
                for blk in range(NBLK):
                    hb = bigpool.tile([P, blkc, D], f32, tag="hids")
                    nc.sync.dma_start(
                        out=hb,
                        in_=hids[b].rearrange("(p n) d -> p n d", n=NCH)[
                            :, blk * blkc : (blk + 1) * blkc, :
                        ],
                    )
                    dummy = small.tile([P, 1], f32, tag="dummy")
                    rawb = pblkpool.tile([P, blkc], f32, tag="rawb")
                    for c in range(blkc):
                        nc.vector.affine_mul_reduce(
                            out=dummy.broadcast_to((P, D)),
                            accum_out=rawb[:, c : c + 1],
                            in0=hb[:, c, :],
                            in1=xb_sb[:, b, :],
                            scale=1.0,
                            bias=0.0,
                        )
                    blk_sl = scores[:, blk * blkc : (blk + 1) * blkc]
                    nc.vector.tensor_add(
                        blk_sl, rawb, segadj[:, b, blk * blkc : (blk + 1) * blkc]
                    )
                    mb = small.tile([P, 1], f32, tag="mb")
                    nc.vector.reduce_max(mb, blk_sl, axis=AxisX)
                    mb_all = small.tile([P, 1], f32, tag="mba")
                    nc.gpsimd.partition_all_reduce(mb_all, mb, P, ReduceOp.max)
                    if blk == 0:
                        m_new = mb_all
                        alpha = None
                    else:
                        m_new = small.tile([P, 1], f32, tag="mnew")
                        nc.vector.tensor_max(m_new, m_cur, mb_all)
                        diff = small.tile([P, 1], f32, tag="diff")
                        nc.vector.tensor_sub(diff, m_cur, m_new)
                        alpha = small.tile([P, 1], f32, tag="alpha")
                        nc.scalar.activation(alpha, diff, Act.Exp)
                    negm = small.tile([P, 1], f32, tag="negm")
                    nc.vector.tensor_scalar_mul(negm, m_new, -1.0)

                    pb = pblkpool.tile([P, blkc], f32, tag="pb")
                    zb = small.tile([P, 1], f32, tag="zb")
                    nc.scalar.activation(
                        pb, blk_sl, Act.Exp, bias=negm, accum_out=zb
                    )
                    zb_all = small.tile([P, 1], f32, tag="zba")
                    nc.gpsimd.partition_all_reduce(zb_all, zb, P, ReduceOp.add)
                    if blk == 0:
                        z_new = zb_all
                    else:
                        z_new = small.tile([P, 1], f32, tag="znew")
                        nc.vector.scalar_tensor_tensor(
                            out=z_new,
                            in0=z_cur,
                            scalar=alpha,
                            in1=zb_all,
                            op0=Alu.mult,
                            op1=Alu.add,
                        )

                    psum_c = psC.tile([1, D], f32, tag="psC")
                    for n in range(NHALF):
                        nsl = slice(n * 512, (n + 1) * 512)
                        if blk > 0:
                            nc.tensor.matmul(
                                psum_c[:, nsl],
                                lhsT=alpha[0:1, 0:1],
                                rhs=c_cur[:, nsl],
                                start=True,
                                stop=False,
                            )
                        for c in range(blkc):
                            nc.tensor.matmul(
                                psum_c[:, nsl],
                                lhsT=pb[:, c : c + 1],
                                rhs=hb[:, c, nsl],
                                start=(blk == 0 and c == 0),
                                stop=(c == blkc - 1),
                            )

                    if blk < NBLK - 1:
                        c_new = cpool.tile([1, D], f32, tag="C")
                        nc.scalar.copy(c_new, psum_c)
                        c_cur = c_new
                    else:
                        # finalize: ctx = C / Z, attn = exp(scores - M - ln Z)
                        rz = small.tile([P, 1], f32, tag="rz")
                        nc.vector.reciprocal(rz, z_new)
                        ctx_sb = outpool.tile([1, D], f32, tag="ctxsb")
                        nc.scalar.activation(
                            ctx_sb, psum_c, Act.Copy, scale=rz[0:1, 0:1]
                        )
                        nc.sync.dma_start(out=ctx_o[b : b + 1, :], in_=ctx_sb)

                        lnz = small.tile([P, 1], f32, tag="lnz")
                        nc.scalar.activation(lnz, z_new, Act.Ln)
                        nbias = small.tile([P, 1], f32, tag="nbias")
                        nc.vector.scalar_tensor_tensor(
                            out=nbias,
                            in0=m_new,
                            scalar=-1.0,
                            in1=lnz,
                            op0=Alu.mult,
                            op1=Alu.subtract,
                        )
                        attn_sb = outpool.tile([P, NCH], f32, tag="attnsb")
                        nc.scalar.activation(attn_sb, scores, Act.Exp, bias=nbias)
                        nc.sync.dma_start(
                            out=attn_o[b].rearrange("(p n) -> p n", n=NCH),
                            in_=attn_sb,
                        )
                    m_cur = m_new
                    z_cur = z_new


def _get_nc(s_total=S, blkc=8):
    key = (s_total, blkc)
    if key not in _NC_CACHE:
        _NC_CACHE[key] = _build(s_total, blkc)
    return _NC_CACHE[key]


def make_in_maps(input, source_hids, seg_scores, W, s_total=S):
    input = np.ascontiguousarray(np.asarray(input, dtype=np.float32))
    source_hids = np.asarray(source_hids, dtype=np.float32)
    seg_scores = np.asarray(seg_scores, dtype=np.float32)
    W = np.asarray(W, dtype=np.float32)
    wt = np.ascontiguousarray(W.T)
    inT = np.ascontiguousarray(input.T)  # [D, B]
    in_maps = []
    for core in range(NCORES):
        sl = slice(core * BLOC, (core + 1) * BLOC)
        in_maps.append(
            {
                "inputT": np.ascontiguousarray(inT[:, sl]),
                "hids": np.ascontiguousarray(source_hids[sl]),
                "seg": np.ascontiguousarray(seg_scores[sl]),
                "WT": wt,
            }
        )
    return in_maps


def kernel(input, source_hids, seg_scores, W):
    global LAST_RESULT
    import os

    from concourse.bass_utils import run_bass_kernel_spmd

    s_total = np.asarray(source_hids).shape[1]
    nc = _get_nc(s_total=s_total)
    in_maps = make_in_maps(input, source_hids, seg_scores, W, s_total=s_total)
    trace = bool(int(os.environ.get("KERNEL_TRACE", "0")))
    res = run_bass_kernel_spmd(
        nc, in_maps, core_ids=list(range(NCORES)), trace=trace
    )
    LAST_RESULT = res
    ctx = np.concatenate([res.results[c]["ctx"] for c in range(NCORES)], axis=0)
    attn = np.concatenate([res.results[c]["attn"] for c in range(NCORES)], axis=0)
    return ctx, attn


# revision 14
# speedup vs baseline: 4323.4121x; 53.8415x over previous
"""Attention layer kernel for Trainium2, data-parallel over batch on 8 NeuronCores.

Reference computation (per full inputs):
    x          = input @ W.T                      [B, D]
    scores     = einsum('bsd,bd->bs', hids, x) - 100*(1-seg)
    attn       = softmax(scores, axis=1)          [B, S]
    ctx        = einsum('bs,bsd->bd', attn, hids) [B, D]
    returns (ctx, attn)

Sharding: batch B=32 split 4-per-core across 8 cores; W replicated
(pre-transposed on host so the contraction dim lands on SBUF partitions).

Per-core algorithm (flash-decoding style, reads source_hids exactly once):
  - x = input @ W^T on TensorE, broadcast to 128 partitions via a DRAM
    round-trip with a stride-0 partition AP.
  - scores per 128-row chunk via one fused DVE tensor_tensor_reduce
    (multiply by broadcast x, reduce over d, init with the -100*(1-seg) mask).
  - online softmax over blocks of chunks: running max M / sum Z, context
    accumulator C rescaled by alpha = exp(M_old - M_new) using a K=1 matmul
    into the same PSUM accumulation group as the per-chunk attn@hids matmuls.
  - s-index mapping s = p*NCH + n keeps every DMA contiguous (no transposes).
"""

import numpy as np

B, S, DIN, DOUT = 32, 4096, 1024, 1024
D = DIN
NCORES = 8
BLOC = B // NCORES  # batches per core
P = 128

_NC_CACHE = {}
LAST_RESULT = None  # BassKernelResults of the most recent run (for profiling)


def _build(s_total=S, blkc=8, reps=1, big_bufs=4):
    """Build + compile the per-core Bass module. Same program on all cores.

    reps > 1 wraps the main computation in an on-device For_i loop — used
    only for timing (slope between reps removes host/RPC overhead).
    """
    import contextlib

    import concourse.bacc as bacc
    import concourse.bass as bass
    import concourse.mybir as mybir
    import concourse.tile as tile
    from concourse.bass_isa import ReduceOp

    f32 = mybir.dt.float32
    Alu = mybir.AluOpType
    Act = mybir.ActivationFunctionType
    AxisX = mybir.AxisListType.X

    NCH = s_total // P          # score chunks per batch (columns of score tile)
    NBLK = NCH // blkc          # online-softmax blocks per batch
    KCH = D // P                # contraction chunks for the x projection
    NHALF = D // 512            # 512-wide matmul column tiles

    nc = bacc.Bacc("TRN2", target_bir_lowering=False, debug=False)

    f32r = mybir.dt.float32r
    inputT = nc.dram_tensor("inputT", [D, BLOC], f32, kind="ExternalInput").ap()
    # hids is declared float32r (same 4-byte payload): the PE reads it
    # natively for single-pass reduced-precision matmuls, while the DVE
    # score pass bitcasts back to float32 and sees the full-precision bits.
    hids = nc.dram_tensor(
        "hids", [BLOC, s_total, D], f32r, kind="ExternalInput"
    ).ap()
    seg = nc.dram_tensor("seg", [BLOC, s_total], f32, kind="ExternalInput").ap()
    WT = nc.dram_tensor("WT", [D, D], f32, kind="ExternalInput").ap()
    ctx_o = nc.dram_tensor("ctx", [BLOC, D], f32, kind="ExternalOutput").ap()
    attn_o = nc.dram_tensor("attn", [BLOC, s_total], f32, kind="ExternalOutput").ap()
    x_dram = nc.dram_tensor("x_scratch", [BLOC, D], f32, kind="Internal").ap()

    with tile.TileContext(nc) as tc:
        with (
            tc.tile_pool(name="big", bufs=big_bufs) as bigpool,
            tc.tile_pool(name="persist", bufs=1) as persist,
            tc.tile_pool(name="scores", bufs=BLOC) as scorepool,
            tc.tile_pool(name="small", bufs=24) as small,
            tc.tile_pool(name="pblk", bufs=4) as pblkpool,
            tc.tile_pool(name="cpool", bufs=4) as cpool,
            tc.tile_pool(name="outp", bufs=2) as outpool,
            tc.tile_pool(name="psA", bufs=1, space="PSUM") as psA,
            tc.tile_pool(name="psC", bufs=2, space="PSUM") as psC,
        ):
            # ---------------- prologue: x = input @ W^T, then broadcast ------
            with tc.tile_pool(name="wpool", bufs=4) as wpool:
                WT_r = WT.rearrange("(k p) o -> p k o", p=P)
                inT_sb = wpool.tile([P, KCH, BLOC], f32, tag="inT")
                nc.sync.dma_start(
                    out=inT_sb, in_=inputT.rearrange("(k p) b -> p k b", p=P)
                )
                psum_x = psA.tile([BLOC, D], f32, tag="psx")
                for k in range(KCH):
                    wt_k = wpool.tile([P, D], f32, tag="WTk")
                    nc.sync.dma_start(out=wt_k, in_=WT_r[:, k, :])
                    for n in range(NHALF):
                        nsl = slice(n * 512, (n + 1) * 512)
                        nc.tensor.matmul(
                            psum_x[:, nsl],
                            lhsT=inT_sb[:, k, :],
                            rhs=wt_k[:, nsl],
                            start=(k == 0),
                            stop=(k == KCH - 1),
                        )
                x_sb = outpool.tile([BLOC, D], f32, tag="x_sb")
                nc.scalar.copy(x_sb, psum_x)
                nc.sync.dma_start(out=x_dram, in_=x_sb)

            # broadcast each batch's x row across all 128 partitions
            xb_sb = persist.tile([P, BLOC, D], f32, tag="xb")
            for b in range(BLOC):
                row = x_dram[b]
                src = bass.AP(
                    tensor=row.tensor,
                    offset=row.offset,
                    ap=[[0, P]] + [list(e) for e in row.ap],
                )
                nc.gpsimd.dma_start(out=xb_sb[:, b, :], in_=src)

            # additive mask term 100*seg - 100, laid out like the score tiles
            segadj = persist.tile([P, BLOC, NCH], f32, tag="segadj")
            for b in range(BLOC):
                seg_raw = outpool.tile([P, NCH], f32, tag="segraw")
                nc.sync.dma_start(
                    out=seg_raw, in_=seg[b].rearrange("(p n) -> p n", n=NCH)
                )
                nc.scalar.activation(
                    segadj[:, b, :], seg_raw, Act.Copy, bias=-100.0, scale=100.0
                )

            # ---------------- main loop: online softmax + context ------------
            rep_cm = tc.For_i(0, reps, 1) if reps > 1 else contextlib.nullcontext()
            with rep_cm:
                _main_body(
                    nc, tc, bass, mybir, ReduceOp,
                    hids, seg, ctx_o, attn_o,
                    xb_sb, segadj,
                    bigpool, scorepool, small, pblkpool, cpool, outpool, psC,
                    s_total, blkc,
                )

    nc.compile()
    return nc


def _main_body(
    nc, tc, bass, mybir, ReduceOp,
    hids, seg, ctx_o, attn_o,
    xb_sb, segadj,
    bigpool, scorepool, small, pblkpool, cpool, outpool, psC,
    s_total, blkc,
):
    f32 = mybir.dt.float32
    f32r = mybir.dt.float32r
    Alu = mybir.AluOpType
    Act = mybir.ActivationFunctionType
    AxisX = mybir.AxisListType.X
    D = DIN
    NCH = s_total // P
    NBLK = NCH // blkc
    NHALF = D // 512
    if True:
            for b in range(BLOC):
                scores = scorepool.tile([P, NCH], f32, tag="scores")
                m_cur = None
                z_cur = None
                c_cur = None
                for blk in range(NBLK):
                    hb = bigpool.tile([P, blkc, D], f32r, tag="hids")
                    nc.sync.dma_start(
                        out=hb,
                        in_=hids[b].rearrange("(p n) d -> p n d", n=NCH)[
                            :, blk * blkc : (blk + 1) * blkc, :
                        ],
                    )
                    dummy = small.tile([P, 1], f32, tag="dummy")
                    rawb = pblkpool.tile([P, blkc], f32, tag="rawb")
                    for c in range(blkc):
                        nc.vector.affine_mul_reduce(
                            out=dummy.broadcast_to((P, D)),
                            accum_out=rawb[:, c : c + 1],
                            in0=hb[:, c, :].bitcast(f32),
                            in1=xb_sb[:, b, :],
                            scale=1.0,
                            bias=0.0,
                        )
                    blk_sl = scores[:, blk * blkc : (blk + 1) * blkc]
                    nc.vector.tensor_add(
                        blk_sl, rawb, segadj[:, b, blk * blkc : (blk + 1) * blkc]
                    )
                    mb = small.tile([P, 1], f32, tag="mb")
                    nc.vector.reduce_max(mb, blk_sl, axis=AxisX)
                    mb_all = small.tile([P, 1], f32, tag="mba")
                    nc.gpsimd.partition_all_reduce(mb_all, mb, P, ReduceOp.max)
                    if blk == 0:
                        m_new = mb_all
                        alpha = None
                    else:
                        m_new = small.tile([P, 1], f32, tag="mnew")
                        nc.vector.tensor_max(m_new, m_cur, mb_all)
                        diff = small.tile([P, 1], f32, tag="diff")
                        nc.vector.tensor_sub(diff, m_cur, m_new)
                        alpha = small.tile([P, 1], f32, tag="alpha")
                        nc.scalar.activation(alpha, diff, Act.Exp)
                        alpha_r = small.tile([P, 1], f32r, tag="alphar")
                        nc.scalar.activation(alpha_r, alpha, Act.Copy)
                    negm = small.tile([P, 1], f32, tag="negm")
                    nc.vector.tensor_scalar_mul(negm, m_new, -1.0)

                    pb = pblkpool.tile([P, blkc], f32r, tag="pb")
                    zb = small.tile([P, 1], f32, tag="zb")
                    nc.scalar.activation(
                        pb, blk_sl, Act.Exp, bias=negm, accum_out=zb
                    )
                    zb_all = small.tile([P, 1], f32, tag="zba")
                    nc.gpsimd.partition_all_reduce(zb_all, zb, P, ReduceOp.add)
                    if blk == 0:
                        z_new = zb_all
                    else:
                        z_new = small.tile([P, 1], f32, tag="znew")
                        nc.vector.scalar_tensor_tensor(
                            out=z_new,
                            in0=z_cur,
                            scalar=alpha,
                            in1=zb_all,
                            op0=Alu.mult,
                            op1=Alu.add,
                        )

                    # float32r operands: single-pass PE matmul (4x faster
                    # than strict fp32); plenty of precision for the context
                    # accumulation (scores stay full f32 on the DVE path).
                    psum_c = psC.tile([1, D], f32, tag="psC")
                    for n in range(NHALF):
                        nsl = slice(n * 512, (n + 1) * 512)
                        if blk > 0:
                            nc.tensor.matmul(
                                psum_c[:, nsl],
                                lhsT=alpha_r[0:1, 0:1],
                                rhs=c_cur[:, nsl],
                                start=True,
                                stop=False,
                            )
                        for c in range(blkc):
                            nc.tensor.matmul(
                                psum_c[:, nsl],
                                lhsT=pb[:, c : c + 1],
                                rhs=hb[:, c, nsl],
                                start=(blk == 0 and c == 0),
                                stop=(c == blkc - 1),
                            )

                    if blk < NBLK - 1:
                        c_new = cpool.tile([1, D], f32r, tag="C")
                        nc.scalar.copy(c_new, psum_c)
                        c_cur = c_new
                    else:
                        # finalize: ctx = C / Z, attn = exp(scores - M - ln Z)
                        rz = small.tile([P, 1], f32, tag="rz")
                        nc.vector.reciprocal(rz, z_new)
                        ctx_sb = outpool.tile([1, D], f32, tag="ctxsb")
                        nc.scalar.activation(
                            ctx_sb, psum_c, Act.Copy, scale=rz[0:1, 0:1]
                        )
                        nc.sync.dma_start(out=ctx_o[b : b + 1, :], in_=ctx_sb)

                        lnz = small.tile([P, 1], f32, tag="lnz")
                        nc.scalar.activation(lnz, z_new, Act.Ln)
                        nbias = small.tile([P, 1], f32, tag="nbias")
                        nc.vector.scalar_tensor_tensor(
                            out=nbias,
                            in0=m_new,
                            scalar=-1.0,
                            in1=lnz,
                            op0=Alu.mult,
                            op1=Alu.subtract,
                        )
                        attn_sb = outpool.tile([P, NCH], f32, tag="attnsb")
                        nc.scalar.activation(attn_sb, scores, Act.Exp, bias=nbias)
                        nc.sync.dma_start(
                            out=attn_o[b].rearrange("(p n) -> p n", n=NCH),
                            in_=attn_sb,
                        )
                    m_cur = m_new
                    z_cur = z_new


def _get_nc(s_total=S, blkc=8):
    key = (s_total, blkc)
    if key not in _NC_CACHE:
        _NC_CACHE[key] = _build(s_total, blkc)
    return _NC_CACHE[key]


def make_in_maps(input, source_hids, seg_scores, W, s_total=S):
    input = np.ascontiguousarray(np.asarray(input, dtype=np.float32))
    source_hids = np.asarray(source_hids, dtype=np.float32)
    seg_scores = np.asarray(seg_scores, dtype=np.float32)
    W = np.asarray(W, dtype=np.float32)
    wt = np.ascontiguousarray(W.T)
    inT = np.ascontiguousarray(input.T)  # [D, B]
    in_maps = []
    for core in range(NCORES):
        sl = slice(core * BLOC, (core + 1) * BLOC)
        in_maps.append(
            {
                "inputT": np.ascontiguousarray(inT[:, sl]),
                "hids": np.ascontiguousarray(source_hids[sl]),
                "seg": np.ascontiguousarray(seg_scores[sl]),
                "WT": wt,
            }
        )
    return in_maps


def kernel(input, source_hids, seg_scores, W):
    global LAST_RESULT
    import os

    from concourse.bass_utils import run_bass_kernel_spmd

    s_total = np.asarray(source_hids).shape[1]
    nc = _get_nc(s_total=s_total)
    in_maps = make_in_maps(input, source_hids, seg_scores, W, s_total=s_total)
    trace = bool(int(os.environ.get("KERNEL_TRACE", "0")))
    res = run_bass_kernel_spmd(
        nc, in_maps, core_ids=list(range(NCORES)), trace=trace
    )
    LAST_RESULT = res
    ctx = np.concatenate([res.results[c]["ctx"] for c in range(NCORES)], axis=0)
    attn = np.concatenate([res.results[c]["attn"] for c in range(NCORES)], axis=0)
    return ctx, attn
